# revision 7
# baseline (speedup 1.0000x reference)
"""Trainium2 Bass kernel for nn_CCM: per-pixel complex 3x3 mask stencil.

Computation (per batch b):
  H_c = m[c] + v1*m[9+c] + v2*m[18+c],  v1/v2 = -1/2 +- i*sqrt(3)/2, c in 0..8
  out(t,f) = sum_c H_c(t,f) * xpad(t + c//3, f + c%3)   (complex)
with xpad zero-padded by 2 rows at the top (causal time) and 1 col each side.

Sharding: pure data-parallel over B=8 across the 8 NeuronCores.

Per-core layout: partition axis = time (tiles of 125 rows, 2 tiles per
block stacked along the free axis -> FD = 2*257), free axis = frequency.
Frequency shifts of x are free-axis AP offsets; the three time shifts are
three partition-offset DMA loads of the padded x planes.

Engine split: taps 0..5 on VectorE, taps 6..8 on GpSimd (separate
accumulators, merged at the end) to balance the two elementwise engines.
"""

import os
import sys

import numpy as np

sys.path.insert(0, "/opt/trn_rl_repo")

B, C, T, F = 8, 27, 1000, 257
TP = 125          # time rows per partition tile
KK = 2            # partition tiles per block (stacked on free axis)
TB = TP * KK      # time rows per block
NBLK = T // TB    # 4
FX = F + 2        # padded freq width
SQ32 = float(np.sqrt(3.0) / 2.0)

_prog_cache = {}


def _build_program():
    import concourse.bass as bass
    import concourse.tile as tile
    from concourse import bacc, mybir

    f32 = mybir.dt.float32
    Alu = mybir.AluOpType

    nc = bacc.Bacc()
    m_d = nc.declare_dram_parameter("m", [C, T, F], f32, isOutput=False)
    xre_d = nc.declare_dram_parameter("xre", [T + 2, FX], f32, isOutput=False)
    xim_d = nc.declare_dram_parameter("xim", [T + 2, FX], f32, isOutput=False)
    ore_d = nc.declare_dram_parameter("outre", [T, F], f32, isOutput=True)
    oim_d = nc.declare_dram_parameter("outim", [T, F], f32, isOutput=True)

    # [blk, kk, p, c, f] view of m (kk split into separate DMAs: 3-dim AP limit)
    m_r = m_d.rearrange("c (blk kk p) f -> blk kk p c f", kk=KK, p=TP)
    ore_r = ore_d.rearrange("(blk kk p) f -> blk p kk f", kk=KK, p=TP)
    oim_r = oim_d.rearrange("(blk kk p) f -> blk p kk f", kk=KK, p=TP)

    with tile.TileContext(nc) as tc:
        from contextlib import ExitStack

        with ExitStack() as ctx:
            mpool = ctx.enter_context(tc.tile_pool(name="mpool", bufs=2))
            xpool = ctx.enter_context(tc.tile_pool(name="xpool", bufs=2))
            hpool = ctx.enter_context(tc.tile_pool(name="hpool", bufs=2))
            ppool = ctx.enter_context(tc.tile_pool(name="ppool", bufs=6))
            opool = ctx.enter_context(tc.tile_pool(name="opool", bufs=2))
            bpool = ctx.enter_context(tc.tile_pool(name="bpool", bufs=1))
            cpool = ctx.enter_context(tc.tile_pool(name="cpool", bufs=1))

            # const tiles for gpsimd taps (Pool lacks tensor_scalar/stt)
            halfc = cpool.tile([TP, KK, F], f32, tag="halfc")
            sqc = cpool.tile([TP, KK, F], f32, tag="sqc")
            nc.gpsimd.memset(halfc, 0.5)
            nc.gpsimd.memset(sqc, SQ32)

            for blk in range(NBLK):
                t0 = blk * TB
                m_t = mpool.tile([TP, C, KK, F], f32)
                for kk in range(KK):
                    nc.sync.dma_start(out=m_t[:, :, kk, :], in_=m_r[blk, kk])

                xs_re = xpool.tile([TP, 3, KK, FX], f32, tag="xsre")
                xs_im = xpool.tile([TP, 3, KK, FX], f32, tag="xsim")
                for d in range(3):
                    src_re = xre_d[t0 + d : t0 + d + TB, :].rearrange(
                        "(kk p) f -> p kk f", p=TP
                    )
                    src_im = xim_d[t0 + d : t0 + d + TB, :].rearrange(
                        "(kk p) f -> p kk f", p=TP
                    )
                    nc.sync.dma_start(out=xs_re[:, d], in_=src_re)
                    nc.sync.dma_start(out=xs_im[:, d], in_=src_im)

                ore_a = opool.tile([TP, KK, F], f32, tag="ore_a")
                oim_a = opool.tile([TP, KK, F], f32, tag="oim_a")
                ore_b = bpool.tile([TP, KK, F], f32, tag="ore_b")
                oim_b = bpool.tile([TP, KK, F], f32, tag="oim_b")

                for c in range(9):
                    mm, nn = divmod(c, 3)
                    eng = nc.vector if c < 6 else nc.gpsimd
                    first = c == 0 or c == 6
                    o_re = ore_a if c < 6 else ore_b
                    o_im = oim_a if c < 6 else oim_b

                    m0 = m_t[:, c]
                    m1 = m_t[:, 9 + c]
                    m2 = m_t[:, 18 + c]
                    xr = xs_re[:, mm, :, nn : nn + F]
                    xi = xs_im[:, mm, :, nn : nn + F]

                    g1 = hpool.tile([TP, KK, F], f32, tag="g1")
                    g2 = hpool.tile([TP, KK, F], f32, tag="g2")
                    hre = hpool.tile([TP, KK, F], f32, tag="hre")
                    eng.tensor_add(g1, m1, m2)
                    eng.tensor_sub(g2, m1, m2)
                    if c < 6:
                        # hre = m0 - 0.5*g1 (fused on DVE)
                        eng.scalar_tensor_tensor(
                            out=hre, in0=g1, scalar=-0.5, in1=m0,
                            op0=Alu.mult, op1=Alu.add,
                        )
                    else:
                        # Pool engine: tensor_tensor only
                        t5 = ppool.tile([TP, KK, F], f32, tag="pv")
                        eng.tensor_mul(t5, g1, halfc)
                        eng.tensor_sub(hre, m0, t5)
                        him = hpool.tile([TP, KK, F], f32, tag="him")
                        eng.tensor_mul(him, g2, sqc)

                    # out_re += hre*xr - (sq32*g2)*xi
                    # out_im += hre*xi + (sq32*g2)*xr
                    if first:
                        eng.tensor_mul(o_re, hre, xr)
                        eng.tensor_mul(o_im, hre, xi)
                    else:
                        p1 = ppool.tile([TP, KK, F], f32, tag="pv")
                        eng.tensor_mul(p1, hre, xr)
                        eng.tensor_add(o_re, o_re, p1)
                        p3 = ppool.tile([TP, KK, F], f32, tag="pv")
                        eng.tensor_mul(p3, hre, xi)
                        eng.tensor_add(o_im, o_im, p3)
                    p2 = ppool.tile([TP, KK, F], f32, tag="pv")
                    if c < 6:
                        eng.scalar_tensor_tensor(
                            out=p2, in0=g2, scalar=SQ32, in1=xi,
                            op0=Alu.mult, op1=Alu.mult,
                        )
                    else:
                        eng.tensor_mul(p2, him, xi)
                    eng.tensor_sub(o_re, o_re, p2)
                    p4 = ppool.tile([TP, KK, F], f32, tag="pv")
                    if c < 6:
                        eng.scalar_tensor_tensor(
                            out=p4, in0=g2, scalar=SQ32, in1=xr,
                            op0=Alu.mult, op1=Alu.mult,
                        )
                    else:
                        eng.tensor_mul(p4, him, xr)
                    eng.tensor_add(o_im, o_im, p4)

                # merge the two accumulator pairs on VectorE
                nc.vector.tensor_add(ore_a, ore_a, ore_b)
                nc.vector.tensor_add(oim_a, oim_a, oim_b)

                nc.sync.dma_start(out=ore_r[blk], in_=ore_a)
                nc.sync.dma_start(out=oim_r[blk], in_=oim_a)

    nc.finalize()
    return nc


def _get_program():
    if "nc" not in _prog_cache:
        _prog_cache["nc"] = _build_program()
    return _prog_cache["nc"]


def _host_prep(m, x):
    in_maps = []
    for b in range(B):
        xb = x[b]  # (F, T, 2)
        xre = np.zeros((T + 2, FX), np.float32)
        xim = np.zeros((T + 2, FX), np.float32)
        xre[2:, 1 : F + 1] = xb[:, :, 0].T
        xim[2:, 1 : F + 1] = xb[:, :, 1].T
        in_maps.append(
            {
                "m": np.ascontiguousarray(m[b]),
                "xre": xre,
                "xim": xim,
            }
        )
    return in_maps


def kernel(m, x, _trace=False):
    from concourse.bass_utils import run_bass_kernel_spmd

    nc = _get_program()
    in_maps = _host_prep(np.asarray(m), np.asarray(x))
    res = run_bass_kernel_spmd(nc, in_maps, list(range(B)), trace=_trace)
    out = np.empty((B, F, T, 2), np.float32)
    for b in range(B):
        out[b, :, :, 0] = res.results[b]["outre"].T
        out[b, :, :, 1] = res.results[b]["outim"].T
    if _trace:
        return out, res
    return out


# revision 9
# speedup vs baseline: 1.6198x; 1.6198x over previous
"""Trainium2 Bass kernel for nn_CCM: per-pixel complex 3x3 mask stencil.

Computation (per batch b):
  H_c = m[c] + v1*m[9+c] + v2*m[18+c],  v1/v2 = -1/2 +- i*sqrt(3)/2, c in 0..8
  out(t,f) = sum_c H_c(t,f) * xpad(t + c//3, f + c%3)   (complex)
with xpad zero-padded by 2 rows at the top (causal time) and 1 col each side.

Sharding: pure data-parallel over B=8 across the 8 NeuronCores.

v2 design:
  - Host packs every DRAM tensor into the exact SBUF tile layout so each
    DMA is one large contiguous descriptor per partition (v1's row-strided
    loads ran at ~110 GB/s on 1KB descriptors).
  - bf16 inputs and products (DVE tensor_tensor 2x mode), fp32 final
    accumulation.  Measured numeric error of this scheme: scale-relative
    max err ~5.7e-3 vs the fp32 reference.
  - partition axis = time (125 rows), KK=4 time-tiles stacked on the free
    axis -> FD = 4*258 per op; 2 blocks cover T=1000.
  - taps 0..7 on VectorE (chained bf16 group sums, two groups of 4),
    tap 8 on GpSimd; merge in fp32 on VectorE.
  - DMAs alternate between the two HWDGE rings (sync + scalar).
"""

import sys

import numpy as np

sys.path.insert(0, "/opt/trn_rl_repo")

B, C, T, F = 8, 27, 1000, 257
FP = F + 1        # padded op width (even element count for bf16 mode)
XW = 260          # x tile width (covers freq shifts 0..2 after variants)
TP = 125          # time rows per partition tile
KK = 4            # time tiles per block, stacked along free axis
TB = TP * KK      # 500 time rows per block
NBLK = T // TB    # 2
SQ32 = float(np.sqrt(3.0) / 2.0)

_prog_cache = {}


def _build_program():
    import concourse.tile as tile
    from concourse import bacc, mybir

    bf16 = mybir.dt.bfloat16
    f32 = mybir.dt.float32
    Alu = mybir.AluOpType

    nc = bacc.Bacc()
    m_d = nc.declare_dram_parameter("m", [NBLK, TP, 9, 3, KK, FP], bf16,
                                    isOutput=False)
    x_d = {}
    for comp in ("re", "im"):
        for off in (0, 1):
            name = f"x{comp}{off}"
            x_d[name] = nc.declare_dram_parameter(
                name, [NBLK, TP, 3, KK, XW], bf16, isOutput=False)
    ore_d = nc.declare_dram_parameter("outre", [NBLK, TP, KK, FP], f32,
                                      isOutput=True)
    oim_d = nc.declare_dram_parameter("outim", [NBLK, TP, KK, FP], f32,
                                      isOutput=True)

    with tile.TileContext(nc) as tc:
        from contextlib import ExitStack

        with ExitStack() as ctx:
            mpool = ctx.enter_context(tc.tile_pool(name="mpool", bufs=10))
            xpool = ctx.enter_context(tc.tile_pool(name="xpool", bufs=2))
            tpool = ctx.enter_context(tc.tile_pool(name="tpool", bufs=10))
            spool = ctx.enter_context(tc.tile_pool(name="spool", bufs=2))
            opool = ctx.enter_context(tc.tile_pool(name="opool", bufs=2))
            gpool = ctx.enter_context(tc.tile_pool(name="gpool", bufs=4))
            cpool = ctx.enter_context(tc.tile_pool(name="cpool", bufs=1))

            # const tiles for the gpsimd tap (Pool lacks tensor_scalar/stt)
            halfc = cpool.tile([TP, KK, FP], bf16, tag="halfc")
            sqc = cpool.tile([TP, KK, FP], bf16, tag="sqc")
            nc.gpsimd.memset(halfc, 0.5)
            nc.gpsimd.memset(sqc, SQ32)

            dma_engines = [nc.sync, nc.scalar]
            ndma = [0]

            def dma(out, in_):
                eng = dma_engines[ndma[0] % 2]
                ndma[0] += 1
                eng.dma_start(out=out, in_=in_)

            for blk in range(NBLK):
                x_t = {}
                for name in ("xre0", "xre1", "xim0", "xim1"):
                    x_t[name] = xpool.tile([TP, 3, KK, XW], bf16, tag=name, name=name)
                    dma(x_t[name], x_d[name][blk])

                m_t = []
                for c in range(9):
                    mt = mpool.tile([TP, 3, KK, FP], bf16, tag="mt", name=f"mt{blk}_{c}")
                    dma(mt, m_d[blk, :, c])
                    m_t.append(mt)

                def xsl(comp, mm, nn):
                    # [TP, KK, FP] slice of the x tile for tap (mm, nn),
                    # 4-byte aligned in all cases
                    if nn == 1:
                        return x_t[f"x{comp}1"][:, mm, :, 0:FP]
                    if nn == 0:
                        return x_t[f"x{comp}0"][:, mm, :, 0:FP]
                    return x_t[f"x{comp}0"][:, mm, :, 2:2 + FP]

                # DVE taps 0..7 in two chained groups; gpsimd tap 8
                s_re = [None, None]
                s_im = [None, None]
                for c in range(8):
                    g = c // 4
                    mm, nn = divmod(c, 3)
                    m0 = m_t[c][:, 0]
                    m1 = m_t[c][:, 1]
                    m2 = m_t[c][:, 2]
                    xr = xsl("re", mm, nn)
                    xi = xsl("im", mm, nn)

                    g1 = tpool.tile([TP, KK, FP], bf16, tag="tv")
                    g2 = tpool.tile([TP, KK, FP], bf16, tag="tv")
                    hre = tpool.tile([TP, KK, FP], bf16, tag="tv")
                    nc.vector.tensor_add(g1, m1, m2)
                    nc.vector.tensor_sub(g2, m1, m2)
                    nc.vector.scalar_tensor_tensor(
                        out=hre, in0=g1, scalar=-0.5, in1=m0,
                        op0=Alu.mult, op1=Alu.add)

                    p1 = tpool.tile([TP, KK, FP], bf16, tag="tv")
                    p2 = tpool.tile([TP, KK, FP], bf16, tag="tv")
                    p3 = tpool.tile([TP, KK, FP], bf16, tag="tv")
                    p4 = tpool.tile([TP, KK, FP], bf16, tag="tv")
                    nc.vector.tensor_mul(p1, hre, xr)
                    nc.vector.scalar_tensor_tensor(
                        out=p2, in0=g2, scalar=SQ32, in1=xi,
                        op0=Alu.mult, op1=Alu.mult)
                    nc.vector.tensor_mul(p3, hre, xi)
                    nc.vector.scalar_tensor_tensor(
                        out=p4, in0=g2, scalar=SQ32, in1=xr,
                        op0=Alu.mult, op1=Alu.mult)

                    if c % 4 == 0:
                        sre = spool.tile([TP, KK, FP], bf16, tag=f"sre{g}")
                        sim_ = spool.tile([TP, KK, FP], bf16, tag=f"sim{g}")
                        s_re[g], s_im[g] = sre, sim_
                        nc.vector.tensor_sub(sre, p1, p2)
                        nc.vector.tensor_add(sim_, p3, p4)
                    else:
                        dre = tpool.tile([TP, KK, FP], bf16, tag="tv")
                        dim_ = tpool.tile([TP, KK, FP], bf16, tag="tv")
                        nc.vector.tensor_sub(dre, p1, p2)
                        nc.vector.tensor_add(dim_, p3, p4)
                        nc.vector.tensor_add(s_re[g], s_re[g], dre)
                        nc.vector.tensor_add(s_im[g], s_im[g], dim_)

                # gpsimd tap 8 (mm=2, nn=2)
                c = 8
                mm, nn = divmod(c, 3)
                m0 = m_t[c][:, 0]
                m1 = m_t[c][:, 1]
                m2 = m_t[c][:, 2]
                xr = xsl("re", mm, nn)
                xi = xsl("im", mm, nn)
                g1 = gpool.tile([TP, KK, FP], bf16, tag="gv")
                g2 = gpool.tile([TP, KK, FP], bf16, tag="gv")
                t5 = gpool.tile([TP, KK, FP], bf16, tag="gv")
                hre = gpool.tile([TP, KK, FP], bf16, tag="gv")
                him = gpool.tile([TP, KK, FP], bf16, tag="gv")
                nc.gpsimd.tensor_add(g1, m1, m2)
                nc.gpsimd.tensor_sub(g2, m1, m2)
                nc.gpsimd.tensor_mul(t5, g1, halfc)
                nc.gpsimd.tensor_sub(hre, m0, t5)
                nc.gpsimd.tensor_mul(him, g2, sqc)
                p1 = gpool.tile([TP, KK, FP], bf16, tag="gv")
                p2 = gpool.tile([TP, KK, FP], bf16, tag="gv")
                p3 = gpool.tile([TP, KK, FP], bf16, tag="gv")
                p4 = gpool.tile([TP, KK, FP], bf16, tag="gv")
                dre8 = spool.tile([TP, KK, FP], bf16, tag="dre8")
                dim8 = spool.tile([TP, KK, FP], bf16, tag="dim8")
                nc.gpsimd.tensor_mul(p1, hre, xr)
                nc.gpsimd.tensor_mul(p2, him, xi)
                nc.gpsimd.tensor_mul(p3, hre, xi)
                nc.gpsimd.tensor_mul(p4, him, xr)
                nc.gpsimd.tensor_sub(dre8, p1, p2)
                nc.gpsimd.tensor_add(dim8, p3, p4)

                # fp32 merges on VectorE
                out_re = opool.tile([TP, KK, FP], f32, tag="out_re")
                out_im = opool.tile([TP, KK, FP], f32, tag="out_im")
                nc.vector.tensor_add(out_re, s_re[0], s_re[1])
                nc.vector.tensor_add(out_re, out_re, dre8)
                nc.vector.tensor_add(out_im, s_im[0], s_im[1])
                nc.vector.tensor_add(out_im, out_im, dim8)

                dma(ore_d[blk], out_re)
                dma(oim_d[blk], out_im)

    nc.finalize()
    return nc


def _get_program():
    if "nc" not in _prog_cache:
        _prog_cache["nc"] = _build_program()
    return _prog_cache["nc"]


def _host_prep(m, x):
    import ml_dtypes

    bf = ml_dtypes.bfloat16
    in_maps = []
    for b in range(B):
        # m[b]: (27, T, F) -> [blk, p, tap, r, kk, f(FP)] bf16
        mb = np.zeros((3, 9, T, FP), np.float32)
        mb[:, :, :, :F] = m[b].reshape(3, 9, T, F)
        mb = mb.reshape(3, 9, NBLK, KK, TP, FP)
        mt = np.ascontiguousarray(
            mb.transpose(2, 4, 1, 0, 3, 5)).astype(bf)

        xb = x[b]  # (F, T, 2)
        planes = {}
        for ci, comp in enumerate(("re", "im")):
            xpad = np.zeros((T + 2, XW + 2), np.float32)
            xpad[2:, 1:F + 1] = xb[:, :, ci].T
            for off in (0, 1):
                v = np.empty((NBLK, TP, 3, KK, XW), np.float32)
                for blk in range(NBLK):
                    for d in range(3):
                        for kk in range(KK):
                            r0 = blk * TB + kk * TP + d
                            v[blk, :, d, kk, :] = xpad[r0:r0 + TP,
                                                       off:off + XW]
                planes[f"x{comp}{off}"] = v.astype(bf)

        in_maps.append({"m": mt, **planes})
    return in_maps


def _assemble(results):
    out = np.empty((B, F, T, 2), np.float32)
    for b in range(B):
        for ci, name in enumerate(("outre", "outim")):
            arr = results[b][name]  # [NBLK, TP, KK, FP]
            full = arr.transpose(0, 2, 1, 3).reshape(T, FP)[:, :F]
            out[b, :, :, ci] = full.T
    return out


def kernel(m, x, _trace=False):
    from concourse.bass_utils import run_bass_kernel_spmd

    nc = _get_program()
    in_maps = _host_prep(np.asarray(m), np.asarray(x))
    res = run_bass_kernel_spmd(nc, in_maps, list(range(B)), trace=_trace)
    out = _assemble(res.results)
    if _trace:
        return out, res
    return out


# revision 10
# speedup vs baseline: 1.9234x; 1.1875x over previous
"""Trainium2 Bass kernel for nn_CCM: per-pixel complex 3x3 mask stencil.

Computation (per batch b):
  H_c = m[c] + v1*m[9+c] + v2*m[18+c],  v1/v2 = -1/2 +- i*sqrt(3)/2, c in 0..8
  out(t,f) = sum_c H_c(t,f) * xpad(t + c//3, f + c%3)   (complex)
with xpad zero-padded by 2 rows at the top (causal time) and 1 col each side.

Sharding: pure data-parallel over B=8 across the 8 NeuronCores.

v3 design (see git-less lineage in kernel_v1/v2.py):
  - Host packs DRAM tensors in SBUF tile order -> one large contiguous
    descriptor per partition per DMA.
  - bf16 inputs/products (DVE 2x), fp32 final accumulation; measured
    numeric error scale-relative ~7.5e-3.
  - m channels 9..26 are pre-scaled by 0.5 on the host (exact power-of-2
    scale folded into the bf16 cast), so the mask needs only add/sub plus
    one tensor_scalar mult by sqrt(3) (4x mode on DVE).
  - taps 0..7 on VectorE (two chained bf16 sum groups), tap 8 on GpSimd;
    fp32 merge on VectorE.  DMAs alternate across both HWDGE rings.
"""

import sys

import numpy as np

sys.path.insert(0, "/opt/trn_rl_repo")

B, C, T, F = 8, 27, 1000, 257
FP = F + 1        # padded op width (even element count for bf16 mode)
XW = 260          # x tile width (covers freq shifts 0..2)
TP = 125          # time rows per partition tile
KK = 4            # time tiles per block, stacked along free axis
TB = TP * KK      # 500 time rows per block
NBLK = T // TB    # 2
SQ3 = float(np.sqrt(3.0))

_prog_cache = {}


def _build_program():
    import concourse.tile as tile
    from concourse import bacc, mybir

    bf16 = mybir.dt.bfloat16
    f32 = mybir.dt.float32

    nc = bacc.Bacc()
    m_d = nc.declare_dram_parameter("m", [NBLK, TP, 9, 3, KK, FP], bf16,
                                    isOutput=False)
    xre_d = nc.declare_dram_parameter("xre", [NBLK, TP, 3, KK, XW], bf16,
                                      isOutput=False)
    xim_d = nc.declare_dram_parameter("xim", [NBLK, TP, 3, KK, XW], bf16,
                                      isOutput=False)
    ore_d = nc.declare_dram_parameter("outre", [NBLK, TP, KK, FP], f32,
                                      isOutput=True)
    oim_d = nc.declare_dram_parameter("outim", [NBLK, TP, KK, FP], f32,
                                      isOutput=True)

    with tile.TileContext(nc) as tc:
        from contextlib import ExitStack

        with ExitStack() as ctx:
            mpool = ctx.enter_context(tc.tile_pool(name="mpool", bufs=10))
            xpool = ctx.enter_context(tc.tile_pool(name="xpool", bufs=2))
            tpool = ctx.enter_context(tc.tile_pool(name="tpool", bufs=10))
            spool = ctx.enter_context(tc.tile_pool(name="spool", bufs=2))
            opool = ctx.enter_context(tc.tile_pool(name="opool", bufs=2))
            gpool = ctx.enter_context(tc.tile_pool(name="gpool", bufs=4))
            cpool = ctx.enter_context(tc.tile_pool(name="cpool", bufs=1))

            # const tile for the gpsimd tap (Pool lacks tensor_scalar)
            sq3c = cpool.tile([TP, KK, FP], bf16, tag="sq3c")
            nc.gpsimd.memset(sq3c, SQ3)

            dma_engines = [nc.sync, nc.scalar]
            ndma = [0]

            def dma(out, in_):
                eng = dma_engines[ndma[0] % 2]
                ndma[0] += 1
                eng.dma_start(out=out, in_=in_)

            for blk in range(NBLK):
                xre_t = xpool.tile([TP, 3, KK, XW], bf16, tag="xre")
                dma(xre_t, xre_d[blk])
                xim_t = xpool.tile([TP, 3, KK, XW], bf16, tag="xim")
                dma(xim_t, xim_d[blk])

                m_t = []
                for c in range(9):
                    mt = mpool.tile([TP, 3, KK, FP], bf16, tag="mt",
                                    name=f"mt{blk}_{c}")
                    dma(mt, m_d[blk, :, c])
                    m_t.append(mt)

                def xsl(xt, mm, nn):
                    # [TP, KK, FP] slice for tap (mm, nn); nn==1 is 2-byte
                    # aligned (runs at 1x) -- cheaper than shipping a
                    # shifted copy through the DMA bottleneck
                    return xt[:, mm, :, nn:nn + FP]

                def tap_ops(eng, c, dre, dim_, tmp):
                    # writes tap c's complex product into dre/dim_ (bf16)
                    mm, nn = divmod(c, 3)
                    m0 = m_t[c][:, 0]
                    m1 = m_t[c][:, 1]   # pre-scaled by 0.5 on host
                    m2 = m_t[c][:, 2]   # pre-scaled by 0.5 on host
                    xr = xsl(xre_t, mm, nn)
                    xi = xsl(xim_t, mm, nn)
                    g1 = tmp()
                    hre = tmp()
                    g2 = tmp()
                    him = tmp()
                    eng.tensor_add(g1, m1, m2)
                    eng.tensor_sub(hre, m0, g1)
                    eng.tensor_sub(g2, m1, m2)
                    if eng is nc.vector:
                        eng.tensor_scalar_mul(him, g2, SQ3)
                    else:
                        eng.tensor_mul(him, g2, sq3c)
                    p1 = tmp()
                    p2 = tmp()
                    p3 = tmp()
                    p4 = tmp()
                    eng.tensor_mul(p1, hre, xr)
                    eng.tensor_mul(p2, him, xi)
                    eng.tensor_mul(p3, hre, xi)
                    eng.tensor_mul(p4, him, xr)
                    eng.tensor_sub(dre, p1, p2)
                    eng.tensor_add(dim_, p3, p4)

                def vtmp():
                    t = tpool.tile([TP, KK, FP], bf16, tag="tv", name="tv")
                    return t

                # DVE taps 0..7 in two chained groups
                s_re = [None, None]
                s_im = [None, None]
                for c in range(8):
                    g = c // 4
                    if c % 4 == 0:
                        s_re[g] = spool.tile([TP, KK, FP], bf16,
                                             tag=f"sre{g}", name=f"sre{g}")
                        s_im[g] = spool.tile([TP, KK, FP], bf16,
                                             tag=f"sim{g}", name=f"sim{g}")
                        tap_ops(nc.vector, c, s_re[g], s_im[g], vtmp)
                    else:
                        dre = vtmp()
                        dim_ = vtmp()
                        tap_ops(nc.vector, c, dre, dim_, vtmp)
                        nc.vector.tensor_add(s_re[g], s_re[g], dre)
                        nc.vector.tensor_add(s_im[g], s_im[g], dim_)

                # gpsimd tap 8
                dre8 = spool.tile([TP, KK, FP], bf16, tag="dre8")
                dim8 = spool.tile([TP, KK, FP], bf16, tag="dim8")

                def gtmp():
                    t = gpool.tile([TP, KK, FP], bf16, tag="gv", name="gv")
                    return t

                tap_ops(nc.gpsimd, 8, dre8, dim8, gtmp)

                # fp32 merges on VectorE
                out_re = opool.tile([TP, KK, FP], f32, tag="out_re")
                out_im = opool.tile([TP, KK, FP], f32, tag="out_im")
                nc.vector.tensor_add(out_re, s_re[0], s_re[1])
                nc.vector.tensor_add(out_re, out_re, dre8)
                nc.vector.tensor_add(out_im, s_im[0], s_im[1])
                nc.vector.tensor_add(out_im, out_im, dim8)

                dma(ore_d[blk], out_re)
                dma(oim_d[blk], out_im)

    nc.finalize()
    return nc


def _get_program():
    if "nc" not in _prog_cache:
        _prog_cache["nc"] = _build_program()
    return _prog_cache["nc"]


def _host_prep(m, x):
    import ml_dtypes

    bf = ml_dtypes.bfloat16
    in_maps = []
    scale = np.array([1.0, 0.5, 0.5], np.float32).reshape(3, 1, 1, 1)
    for b in range(B):
        # m[b]: (27, T, F) -> [blk, p, tap, r, kk, f(FP)] bf16; r=1,2 halved
        mb = np.zeros((3, 9, T, FP), np.float32)
        mb[:, :, :, :F] = m[b].reshape(3, 9, T, F)
        mb *= scale
        mb = mb.reshape(3, 9, NBLK, KK, TP, FP)
        mt = np.ascontiguousarray(mb.transpose(2, 4, 1, 0, 3, 5)).astype(bf)

        xb = x[b]  # (F, T, 2)
        planes = {}
        for ci, name in ((0, "xre"), (1, "xim")):
            xpad = np.zeros((T + 2, XW + 1), np.float32)
            xpad[2:, 1:F + 1] = xb[:, :, ci].T
            v = np.empty((NBLK, TP, 3, KK, XW), np.float32)
            for blk in range(NBLK):
                for d in range(3):
                    for kk in range(KK):
                        r0 = blk * TB + kk * TP + d
                        v[blk, :, d, kk, :] = xpad[r0:r0 + TP, 0:XW]
            planes[name] = v.astype(bf)

        in_maps.append({"m": mt, **planes})
    return in_maps


def _assemble(results):
    out = np.empty((B, F, T, 2), np.float32)
    for b in range(B):
        for ci, name in enumerate(("outre", "outim")):
            arr = results[b][name]  # [NBLK, TP, KK, FP]
            full = arr.transpose(0, 2, 1, 3).reshape(T, FP)[:, :F]
            out[b, :, :, ci] = full.T
    return out


def kernel(m, x, _trace=False):
    from concourse.bass_utils import run_bass_kernel_spmd

    nc = _get_program()
    in_maps = _host_prep(np.asarray(m), np.asarray(x))
    res = run_bass_kernel_spmd(nc, in_maps, list(range(B)), trace=_trace)
    out = _assemble(res.results)
    if _trace:
        return out, res
    return out
